# revision 22
# baseline (speedup 1.0000x reference)
"""DigitMoE forward on 8 Trainium2 NeuronCores — pure data parallel.

Model (per reference):
  h1 = relu(x @ W1[e] + b1[e])          x:[B,784]  -> [B,E,256]
  h2 = relu(h1 @ W2[e] + b2[e])                     -> [B,E,64]
  eo = h2 . W3[e] + b3[e]                           -> [B,E]
  g  = relu(x @ Wg1 + bg1); gs = softmax(g @ Wg2 + bg2)
  out = gs * eo;  returns (out, gs, eo)

Kernel strategy (per core, B_c = B/8 = 8192):
  * All big matmuls in float32r (TF32-like, ~1e-4 rel err, streams at
    ~1 cyc/col like bf16; fp32 is 4x slower).
  * Weights resident in SBUF; gate layer 1 treated as an 11th expert.
  * x is transposed on host -> xT [784, B_c]; streamed per 512-batch chunk.
  * Everything on chip stays [feature, batch]; outputs are written to DRAM
    as [10, B_c] and transposed back on host.
  * MM1 contraction split 6x128 + 16-row remainder; the remainder matmuls of
    the 4 interleaved chains sit in distinct 32-row tile positions. MM1 runs
    4 PSUM accumulation chains (2 units x 2 output halves) interleaved to
    hide PE weight-load/drain latency.
  * expert_outputs: W3 stored as [64, 10] blocks with only column e
    nonzero, so all 10 rank-64 matmuls accumulate into one [10,512] PSUM
    tile (no partition gathers).
  * softmax over 10 experts via exp (bias fused in ACT), partition-sum with
    a ones [10,1] matmul, reciprocal on DVE, broadcast back with a ones
    [1,10] matmul.
"""
import numpy as np

import concourse.bass as bass
import concourse.tile as tile
from concourse import bacc, mybir
from concourse.bass_utils import run_bass_kernel_spmd

F32 = mybir.dt.float32
F32R = mybir.dt.float32r
AF = mybir.ActivationFunctionType
ALU = mybir.AluOpType

E, D_IN, H1, H2 = 10, 784, 256, 64
B = 65536
NCORES = 8
BC = B // NCORES          # 8192 batch rows per core
NB = BC // 512            # 16 batch chunks
NBC = 512                 # batch columns per chunk
KC1, KP1 = 6, 128         # MM1 contraction: 6 chunks of 128 + 16-row remainder
KR = D_IN - KC1 * KP1     # 16
# chain position within its 4-chain emission group, for remainder row-packing
_GROUPS = [[E, 0], [1, 2], [3, 4], [5, 6], [7, 8], [9]]
CHAINPOS = {}
for _g in _GROUPS:
    for _i, _u in enumerate(_g):
        for _mc in range(2):
            CHAINPOS[(_u, _mc)] = _i * 2 + _mc
NU = E + 1                # 11 units: 10 experts + gate layer 1

_CACHE = {}


def _build(nb=NB, reps=1, timing=False):
    nc = bacc.Bacc("TRN2", target_bir_lowering=False, debug=False,
                   num_devices=NCORES)
    CW = KC1 * NBC + NBC      # 3584 packed cols per chunk (main + remainder)
    xpk_cols = CW if timing else NB * CW
    xT_d = nc.dram_tensor("xpk", [128, xpk_cols], F32R, kind="ExternalInput").ap()
    w1_d = nc.dram_tensor("w1p", [KP1, KC1 * NU * H1], F32R, kind="ExternalInput").ap()
    w1r_d = nc.dram_tensor("w1r", [128, NU * 2 * 128], F32R, kind="ExternalInput").ap()
    b1_d = nc.dram_tensor("b1p", [128, NU * 2], F32, kind="ExternalInput").ap()
    w2_d = nc.dram_tensor("w2p", [128, E * 2 * H2], F32R, kind="ExternalInput").ap()
    b2_d = nc.dram_tensor("b2p", [H2, E], F32, kind="ExternalInput").ap()
    w3_d = nc.dram_tensor("w3p", [H2, E * E], F32R, kind="ExternalInput").ap()
    b3_d = nc.dram_tensor("b3p", [E, 1], F32, kind="ExternalInput").ap()
    wg2_d = nc.dram_tensor("wg2p", [128, 2 * E], F32R, kind="ExternalInput").ap()
    bg2_d = nc.dram_tensor("bg2p", [E, 1], F32, kind="ExternalInput").ap()
    ones10_d = nc.dram_tensor("ones10", [E, 1], F32R, kind="ExternalInput").ap()
    ones1x10_d = nc.dram_tensor("ones1x10", [1, E], F32R, kind="ExternalInput").ap()
    out_d = nc.dram_tensor("out_T", [E, BC], F32, kind="ExternalOutput").ap()
    gs_d = nc.dram_tensor("gs_T", [E, BC], F32, kind="ExternalOutput").ap()
    eo_d = nc.dram_tensor("eo_T", [E, BC], F32, kind="ExternalOutput").ap()

    with tile.TileContext(nc) as tc:
        with (
            tc.tile_pool(name="wp", bufs=1) as wp,
            tc.tile_pool(name="xp", bufs=3) as xp,
            tc.tile_pool(name="hp", bufs=5) as hp,
            tc.tile_pool(name="h2p", bufs=3) as h2p,
            tc.tile_pool(name="tp", bufs=2) as tp,
            tc.tile_pool(name="mm1", bufs=4, space="PSUM") as pmm1,
            tc.tile_pool(name="mm2", bufs=2, space="PSUM") as pmm2,
            tc.tile_pool(name="peo", bufs=1, space="PSUM") as peo,
            tc.tile_pool(name="pgt", bufs=1, space="PSUM") as pgt,
        ):
            # ---- resident weights ----
            w1s = wp.tile([KP1, KC1 * NU * H1], F32R, tag="w1s")
            w1r = wp.tile([128, NU * 2 * 128], F32R, tag="w1r")
            b1s = wp.tile([128, NU * 2], F32, tag="b1s")
            w2s = wp.tile([128, E * 2 * H2], F32R, tag="w2s")
            b2s = wp.tile([H2, E], F32, tag="b2s")
            w3s = wp.tile([H2, E * E], F32R, tag="w3s")
            b3s = wp.tile([E, 1], F32, tag="b3s")
            wg2s = wp.tile([128, 2 * E], F32R, tag="wg2s")
            bg2s = wp.tile([E, 1], F32, tag="bg2s")
            ones10 = wp.tile([E, 1], F32R, tag="ones10t")
            ones1x10 = wp.tile([1, E], F32R, tag="ones1x10t")
            nc.sync.dma_start(w1s[:], w1_d)
            nc.sync.dma_start(w1r[:], w1r_d)
            nc.sync.dma_start(b1s[:], b1_d)
            nc.sync.dma_start(w2s[:], w2_d)
            nc.sync.dma_start(b2s[:], b2_d)
            nc.sync.dma_start(w3s[:], w3_d)
            nc.sync.dma_start(b3s[:], b3_d)
            nc.sync.dma_start(wg2s[:], wg2_d)
            nc.sync.dma_start(bg2s[:], bg2_d)
            nc.sync.dma_start(ones10[:], ones10_d)
            nc.sync.dma_start(ones1x10[:], ones1x10_d)

            def w1_ap(kc, u, mc):
                c = (kc * NU + u) * H1 + mc * 128
                return w1s[:, c:c + 128]

            from contextlib import ExitStack as _ES
            _stk = _ES()
            if reps > 1:
                _stk.enter_context(tc.For_i(0, reps, 1))
            prev_tail = None
            for bi in range(nb):
                # ---- stream xT chunk: 7 x [112, 512] ----
                xts = xp.tile([128, CW], F32R, tag="xts")
                bcol = 0 if timing else bi
                nc.sync.dma_start(xts[:], xT_d[:, bass.ts(bcol, CW)])
                xr = xts[:, KC1 * NBC:CW]

                # MM1 in 4-way interleaved chains (two units at a time);
                # MM2/MM3 are skewed behind MM1 of later experts; MM3 is 10
                # sparse-stationary matmuls accumulating one [10,512] PSUM
                # tile. The previous chunk's gate tail is emitted early in
                # this chunk so the in-order PE never waits on ACT/DVE
                # latency.
                h_tiles = {}
                h2_tiles = {}
                eoacc = peo.tile([E, NBC], F32, tag="eoacc")

                def mm1group(units):
                    chains = [(u, mc) for u in units for mc in range(2)]
                    accs = []
                    for _ in chains:
                        a = pmm1.tile([128, NBC], F32, tag="hacc")
                        accs.append(a)
                    for kc in range(KC1):
                        for j, (u, mc) in enumerate(chains):
                            nc.tensor.matmul(
                                accs[j][:], w1_ap(kc, u, mc),
                                xts[:, bass.ts(kc, NBC)],
                                start=(kc == 0), stop=False)
                    for j, (u, mc) in enumerate(chains):
                        cp = CHAINPOS[(u, mc)]
                        col = (u * 2 + mc) * 128
                        nc.tensor.matmul(
                            accs[j][:],
                            w1r[32 * cp:32 * cp + KR, col:col + 128],
                            xr[32 * cp:32 * cp + KR, :],  # packed remainder block
                            start=False, stop=True,
                            tile_position=(32 * cp, 0))
                    for u in units:
                        tag = "g" if u == E else "h"
                        ht = (tp if u == E else hp).tile(
                            [128, 2 * NBC], F32R, tag=tag)
                        h_tiles[u] = ht
                    for j, (u, mc) in enumerate(chains):
                        if mc == 0:
                            nc.vector.tensor_scalar(
                                h_tiles[u][:, bass.ts(mc, NBC)], accs[j][:],
                                b1s[:, u * 2 + mc:u * 2 + mc + 1], 0.0,
                                op0=ALU.add, op1=ALU.max)
                        else:
                            nc.scalar.activation(
                                h_tiles[u][:, bass.ts(mc, NBC)], accs[j][:],
                                AF.Relu,
                                bias=b1s[:, u * 2 + mc:u * 2 + mc + 1])

                def mm2(e):
                    ht = h_tiles.pop(e)
                    h2acc = pmm2.tile([H2, NBC], F32, tag="h2acc")
                    for kc in range(2):
                        nc.tensor.matmul(
                            h2acc[:],
                            w2s[:, (e * 2 + kc) * H2:(e * 2 + kc + 1) * H2],
                            ht[:, bass.ts(kc, NBC)],
                            start=(kc == 0), stop=(kc == 1))
                    h2t = h2p.tile([H2, NBC], F32R, tag="h2t")
                    nc.vector.tensor_scalar(
                        h2t[:], h2acc[:], b2s[:, e:e + 1], 0.0,
                        op0=ALU.add, op1=ALU.max)
                    h2_tiles[e] = h2t

                def mm3(e, eoacc=eoacc):
                    h2t = h2_tiles.pop(e)
                    nc.tensor.matmul(
                        eoacc[:], w3s[:, e * E:(e + 1) * E], h2t[:],
                        start=(e == 0), stop=(e == E - 1))

                def make_tail(bi, gt, eoacc):
                    # Three stages, emitted between the NEXT chunk's MM1
                    # groups so the in-order PE never reaches a matmul whose
                    # ACT/DVE producer (exp, reciprocal) hasn't had a full
                    # MM1 group (~6us) to finish.
                    state = {}

                    def tail1():
                        zacc = pgt.tile([E, NBC], F32, tag="gtail")
                        for kc in range(2):
                            nc.tensor.matmul(
                                zacc[:], wg2s[:, bass.ts(kc, E)],
                                gt[:, bass.ts(kc, NBC)],
                                start=(kc == 0), stop=(kc == 1))
                        ez = tp.tile([E, NBC], F32R, tag="ez")
                        nc.scalar.activation(ez[:], zacc[:], AF.Exp, bias=bg2s[:])
                        state["ez"] = ez

                    def tail2():
                        ez = state["ez"]
                        ssum = pgt.tile([1, NBC], F32, tag="gtail")
                        nc.tensor.matmul(ssum[:], ones10[:], ez[:],
                                         start=True, stop=True)
                        rs = tp.tile([1, NBC], F32R, tag="rs")
                        with nc.allow_low_precision(reason="f32r softmax recip"):
                            nc.vector.reciprocal(rs[:], ssum[:])
                        state["rs"] = rs

                    def tail3():
                        ez, rs = state["ez"], state["rs"]
                        rbacc = pgt.tile([E, NBC], F32, tag="gtail")
                        nc.tensor.matmul(rbacc[:], ones1x10[:], rs[:],
                                         start=True, stop=True)
                        gst = tp.tile([E, NBC], F32, tag="gst")
                        nc.vector.tensor_mul(gst[:], ez[:].bitcast(F32), rbacc[:])
                        eot = tp.tile([E, NBC], F32, tag="eot")
                        nc.vector.tensor_scalar(eot[:], eoacc[:], b3s[:], None,
                                                op0=ALU.add)
                        outt = tp.tile([E, NBC], F32, tag="outt")
                        nc.vector.tensor_mul(outt[:], gst[:], eot[:])
                        nc.sync.dma_start(gs_d[:, bass.ts(bi, NBC)], gst[:])
                        nc.sync.dma_start(eo_d[:, bass.ts(bi, NBC)], eot[:])
                        nc.sync.dma_start(out_d[:, bass.ts(bi, NBC)], outt[:])

                    return (tail1, tail2, tail3)

                mm1group([E, 0])      # gate first so g is ready early
                if prev_tail is not None:
                    prev_tail[0]()
                mm1group([1, 2])
                if prev_tail is not None:
                    prev_tail[1]()
                mm2(0)
                mm1group([3, 4])
                if prev_tail is not None:
                    prev_tail[2]()
                mm2(1)
                mm2(2)
                mm3(0)
                mm1group([5, 6])
                mm2(3)
                mm3(1)
                mm2(4)
                mm3(2)
                mm1group([7, 8])
                mm2(5)
                mm3(3)
                mm2(6)
                mm3(4)
                mm1group([9])
                mm2(7)
                mm3(5)
                mm2(8)
                mm3(6)
                mm2(9)
                mm3(7)
                mm3(8)
                mm3(9)
                prev_tail = make_tail(bi, h_tiles.pop(E), eoacc)
                if bi == nb - 1:
                    prev_tail[0]()
                    prev_tail[1]()
                    prev_tail[2]()
                    prev_tail = None
            _stk.close()

    nc.compile()
    return nc


def _pack_weights(W1, b1, W2, b2, W3, b3, Wg1, bg1, Wg2, bg2):
    f32 = np.float32
    W1g = np.concatenate([W1, Wg1[None]], axis=0).astype(f32)      # [11,784,256]
    b1g = np.concatenate([b1, bg1[None]], axis=0).astype(f32)      # [11,256]
    w1p = np.ascontiguousarray(
        W1g[:, :KC1 * KP1].reshape(NU, KC1, KP1, H1)
        .transpose(2, 1, 0, 3).reshape(KP1, -1))
    w1r = np.zeros((128, NU * 2 * 128), f32)
    for u in range(NU):
        for mc in range(2):
            cp = CHAINPOS[(u, mc)]
            w1r[32 * cp:32 * cp + KR, (u * 2 + mc) * 128:(u * 2 + mc + 1) * 128] = \
                W1g[u, KC1 * KP1:, mc * 128:(mc + 1) * 128]
    b1p = np.ascontiguousarray(
        b1g.reshape(NU, 2, 128).transpose(2, 0, 1).reshape(128, NU * 2))
    w2p = np.ascontiguousarray(
        W2.astype(f32).reshape(E, 2, 128, H2).transpose(2, 0, 1, 3).reshape(128, -1))
    b2p = np.ascontiguousarray(b2.astype(f32).T)                   # [64,10]
    # sparse W3: block e is [64,10] with only column e nonzero
    w3p = np.zeros((H2, E * E), f32)
    for e in range(E):
        w3p[:, e * E + e] = W3[e]
    b3p = b3.astype(f32)[:, None]
    wg2p = np.ascontiguousarray(
        Wg2.astype(f32).reshape(2, 128, E).transpose(1, 0, 2).reshape(128, 2 * E))
    bg2p = bg2.astype(f32)[:, None]
    return {
        "w1p": w1p, "w1r": w1r, "b1p": b1p, "w2p": w2p, "b2p": b2p, "w3p": w3p,
        "b3p": b3p, "wg2p": wg2p, "bg2p": bg2p,
        "ones10": np.ones((E, 1), f32), "ones1x10": np.ones((1, E), f32),
    }


def _pack_x(xs, timing=False):
    """xs [BC, 784] f32 -> [128, NB*3584] chunk-major packed layout.

    Per chunk: 6 main K-blocks [128,512] then the 16 remainder rows
    replicated at partition offsets 0/32/64/96 (matching CHAINPOS groups).
    """
    nb = 1 if timing else NB
    xs = xs[:nb * NBC] if timing else xs
    main = np.ascontiguousarray(
        xs[:, :KC1 * KP1].reshape(nb, NBC, KC1, 128).transpose(3, 0, 2, 1))
    rem_src = xs[:, KC1 * KP1:].reshape(nb, NBC, KR).transpose(2, 0, 1)
    rem = np.zeros((128, nb, NBC), np.float32)
    for g in range(4):
        rem[32 * g:32 * g + KR] = rem_src
    full = np.concatenate(
        [main.reshape(128, nb, KC1 * NBC), rem], axis=2)
    return np.ascontiguousarray(full.reshape(128, nb * (KC1 * NBC + NBC)))


def kernel(x, W1, b1, W2, b2, W3, b3, Wg1, bg1, Wg2, bg2, _nb=NB):
    if _nb not in _CACHE:
        _CACHE[_nb] = _build(_nb)
    nc = _CACHE[_nb]

    args = [np.asarray(a, np.float32)
            for a in (W1, b1, W2, b2, W3, b3, Wg1, bg1, Wg2, bg2)]
    wmap = _pack_weights(*args)
    x = np.asarray(x, np.float32)
    in_maps = []
    for c in range(NCORES):
        in_maps.append({"xpk": _pack_x(x[c * BC:(c + 1) * BC]), **wmap})

    res = None
    for attempt in range(3):
        try:
            res = run_bass_kernel_spmd(nc, in_maps, list(range(NCORES))).results
            break
        except Exception:
            if attempt == 2:
                raise
            import time as _t
            _t.sleep(2.0)
    out = np.concatenate([res[c]["out_T"].T for c in range(NCORES)], axis=0)
    gs = np.concatenate([res[c]["gs_T"].T for c in range(NCORES)], axis=0)
    eo = np.concatenate([res[c]["eo_T"].T for c in range(NCORES)], axis=0)
    return (out, gs, eo)


# revision 23
# speedup vs baseline: 1.2763x; 1.2763x over previous
"""DigitMoE forward on 8 Trainium2 NeuronCores — pure data parallel.

Model (per reference):
  h1 = relu(x @ W1[e] + b1[e])          x:[B,784]  -> [B,E,256]
  h2 = relu(h1 @ W2[e] + b2[e])                     -> [B,E,64]
  eo = h2 . W3[e] + b3[e]                           -> [B,E]
  g  = relu(x @ Wg1 + bg1); gs = softmax(g @ Wg2 + bg2)
  out = gs * eo;  returns (out, gs, eo)

Kernel strategy (per core, B_c = B/8 = 8192):
  * All big matmuls in float32r (TF32-like, ~1e-4 rel err, streams at
    ~1 cyc/col like bf16; fp32 is 4x slower).
  * Weights resident in SBUF; gate layer 1 treated as an 11th expert.
  * x is transposed on host -> xT [784, B_c]; streamed per 512-batch chunk.
  * Everything on chip stays [feature, batch]; outputs are written to DRAM
    as [10, B_c] and transposed back on host.
  * MM1 contraction split 6x128 + 16-row remainder; the remainder matmuls of
    the 4 interleaved chains sit in distinct 32-row tile positions. MM1 runs
    4 PSUM accumulation chains (2 units x 2 output halves) interleaved to
    hide PE weight-load/drain latency.
  * expert_outputs: W3 stored as [64, 10] blocks with only column e
    nonzero, so all 10 rank-64 matmuls accumulate into one [10,512] PSUM
    tile (no partition gathers).
  * softmax over 10 experts via exp (bias fused in ACT), partition-sum with
    a ones [10,1] matmul, reciprocal on DVE, broadcast back with a ones
    [1,10] matmul.
"""
import numpy as np

import concourse.bass as bass
import concourse.tile as tile
from concourse import bacc, mybir
from concourse.bass_utils import run_bass_kernel_spmd

F32 = mybir.dt.float32
F32R = mybir.dt.float32r
AF = mybir.ActivationFunctionType
ALU = mybir.AluOpType

E, D_IN, H1, H2 = 10, 784, 256, 64
B = 65536
NCORES = 8
BC = B // NCORES          # 8192 batch rows per core
NB = BC // 512            # 16 batch chunks
NBC = 512                 # batch columns per chunk
KC1, KP1 = 6, 128         # MM1 contraction: 6 chunks of 128 + 16-row remainder
KR = D_IN - KC1 * KP1     # 16
# chain position within its 4-chain emission group, for remainder row-packing
_GROUPS = [[E, 0], [1, 2], [3, 4], [5, 6], [7, 8], [9]]
CHAINPOS = {}
for _g in _GROUPS:
    for _i, _u in enumerate(_g):
        for _mc in range(2):
            CHAINPOS[(_u, _mc)] = _i * 2 + _mc
NU = E + 1                # 11 units: 10 experts + gate layer 1

_CACHE = {}


def _build(nb=NB, reps=1, timing=False):
    nc = bacc.Bacc("TRN2", target_bir_lowering=False, debug=False,
                   num_devices=NCORES)
    CW = KC1 * NBC + NBC      # 3584 packed cols per chunk (main + remainder)
    xpk_cols = CW if timing else NB * CW
    xT_d = nc.dram_tensor("xpk", [128, xpk_cols], F32R, kind="ExternalInput").ap()
    w1_d = nc.dram_tensor("w1p", [KP1, KC1 * NU * H1], F32R, kind="ExternalInput").ap()
    w1r_d = nc.dram_tensor("w1r", [128, NU * 2 * 128], F32R, kind="ExternalInput").ap()
    b1_d = nc.dram_tensor("b1p", [128, NU * 2], F32, kind="ExternalInput").ap()
    w2_d = nc.dram_tensor("w2p", [128, E * 2 * H2], F32R, kind="ExternalInput").ap()
    b2_d = nc.dram_tensor("b2p", [H2, E], F32, kind="ExternalInput").ap()
    w3_d = nc.dram_tensor("w3p", [H2, E * E], F32R, kind="ExternalInput").ap()
    b3_d = nc.dram_tensor("b3p", [E, 1], F32, kind="ExternalInput").ap()
    wg2_d = nc.dram_tensor("wg2p", [128, 2 * E], F32R, kind="ExternalInput").ap()
    bg2_d = nc.dram_tensor("bg2p", [E, 1], F32, kind="ExternalInput").ap()
    ones10_d = nc.dram_tensor("ones10", [E, 1], F32R, kind="ExternalInput").ap()
    ones1x10_d = nc.dram_tensor("ones1x10", [1, E], F32R, kind="ExternalInput").ap()
    out_d = nc.dram_tensor("out_T", [E, BC], F32, kind="ExternalOutput").ap()
    gs_d = nc.dram_tensor("gs_T", [E, BC], F32, kind="ExternalOutput").ap()
    eo_d = nc.dram_tensor("eo_T", [E, BC], F32, kind="ExternalOutput").ap()

    with tile.TileContext(nc) as tc:
        with (
            tc.tile_pool(name="wp", bufs=1) as wp,
            tc.tile_pool(name="xp", bufs=3) as xp,
            tc.tile_pool(name="hp", bufs=5) as hp,
            tc.tile_pool(name="h2p", bufs=3) as h2p,
            tc.tile_pool(name="tp", bufs=2) as tp,
            tc.tile_pool(name="mm1", bufs=4, space="PSUM") as pmm1,
            tc.tile_pool(name="mm2", bufs=2, space="PSUM") as pmm2,
            tc.tile_pool(name="peo", bufs=1, space="PSUM") as peo,
            tc.tile_pool(name="pgt", bufs=1, space="PSUM") as pgt,
        ):
            # ---- resident weights ----
            w1s = wp.tile([KP1, KC1 * NU * H1], F32R, tag="w1s")
            w1r = wp.tile([128, NU * 2 * 128], F32R, tag="w1r")
            b1s = wp.tile([128, NU * 2], F32, tag="b1s")
            w2s = wp.tile([128, E * 2 * H2], F32R, tag="w2s")
            b2s = wp.tile([H2, E], F32, tag="b2s")
            w3s = wp.tile([H2, E * E], F32R, tag="w3s")
            b3s = wp.tile([E, 1], F32, tag="b3s")
            wg2s = wp.tile([128, 2 * E], F32R, tag="wg2s")
            bg2s = wp.tile([E, 1], F32, tag="bg2s")
            ones10 = wp.tile([E, 1], F32R, tag="ones10t")
            ones1x10 = wp.tile([1, E], F32R, tag="ones1x10t")
            nc.sync.dma_start(b1s[:], b1_d)
            nc.sync.dma_start(b2s[:], b2_d)
            nc.sync.dma_start(b3s[:], b3_d)
            nc.sync.dma_start(bg2s[:], bg2_d)
            nc.sync.dma_start(ones10[:], ones10_d)
            nc.sync.dma_start(ones1x10[:], ones1x10_d)
            nc.sync.dma_start(w2s[:], w2_d)
            nc.sync.dma_start(w3s[:], w3_d)
            nc.sync.dma_start(wg2s[:], wg2_d)
            _W1BLK = NU * H1
            for _kc in range(KC1):
                nc.sync.dma_start(
                    w1s[:, _kc * _W1BLK:(_kc + 1) * _W1BLK],
                    w1_d[:, _kc * _W1BLK:(_kc + 1) * _W1BLK])
            nc.sync.dma_start(w1r[:], w1r_d)

            def w1_ap(kc, u, mc):
                c = (kc * NU + u) * H1 + mc * 128
                return w1s[:, c:c + 128]

            from contextlib import ExitStack as _ES
            _stk = _ES()
            if reps > 1:
                _stk.enter_context(tc.For_i(0, reps, 1))
            prev_tail = None
            for bi in range(nb):
                # ---- stream xT chunk: 7 x [112, 512] ----
                xts = xp.tile([128, CW], F32R, tag="xts")
                bcol = 0 if timing else bi
                nc.sync.dma_start(xts[:], xT_d[:, bass.ts(bcol, CW)])
                xr = xts[:, KC1 * NBC:CW]

                # MM1 in 4-way interleaved chains (two units at a time);
                # MM2/MM3 are skewed behind MM1 of later experts; MM3 is 10
                # sparse-stationary matmuls accumulating one [10,512] PSUM
                # tile. The previous chunk's gate tail is emitted early in
                # this chunk so the in-order PE never waits on ACT/DVE
                # latency.
                h_tiles = {}
                h2_tiles = {}
                eoacc = peo.tile([E, NBC], F32, tag="eoacc")

                def mm1group(units):
                    chains = [(u, mc) for u in units for mc in range(2)]
                    accs = []
                    for _ in chains:
                        a = pmm1.tile([128, NBC], F32, tag="hacc")
                        accs.append(a)
                    for kc in range(KC1):
                        for j, (u, mc) in enumerate(chains):
                            nc.tensor.matmul(
                                accs[j][:], w1_ap(kc, u, mc),
                                xts[:, bass.ts(kc, NBC)],
                                start=(kc == 0), stop=False)
                    for j, (u, mc) in enumerate(chains):
                        cp = CHAINPOS[(u, mc)]
                        col = (u * 2 + mc) * 128
                        nc.tensor.matmul(
                            accs[j][:],
                            w1r[32 * cp:32 * cp + KR, col:col + 128],
                            xr[32 * cp:32 * cp + KR, :],  # packed remainder block
                            start=False, stop=True,
                            tile_position=(32 * cp, 0))
                    for u in units:
                        tag = "g" if u == E else "h"
                        ht = (tp if u == E else hp).tile(
                            [128, 2 * NBC], F32R, tag=tag)
                        h_tiles[u] = ht
                    for j, (u, mc) in enumerate(chains):
                        if mc == 0:
                            nc.vector.tensor_scalar(
                                h_tiles[u][:, bass.ts(mc, NBC)], accs[j][:],
                                b1s[:, u * 2 + mc:u * 2 + mc + 1], 0.0,
                                op0=ALU.add, op1=ALU.max)
                        else:
                            nc.scalar.activation(
                                h_tiles[u][:, bass.ts(mc, NBC)], accs[j][:],
                                AF.Relu,
                                bias=b1s[:, u * 2 + mc:u * 2 + mc + 1])

                def mm2(e):
                    ht = h_tiles.pop(e)
                    h2acc = pmm2.tile([H2, NBC], F32, tag="h2acc")
                    for kc in range(2):
                        nc.tensor.matmul(
                            h2acc[:],
                            w2s[:, (e * 2 + kc) * H2:(e * 2 + kc + 1) * H2],
                            ht[:, bass.ts(kc, NBC)],
                            start=(kc == 0), stop=(kc == 1))
                    h2t = h2p.tile([H2, NBC], F32R, tag="h2t")
                    nc.vector.tensor_scalar(
                        h2t[:], h2acc[:], b2s[:, e:e + 1], 0.0,
                        op0=ALU.add, op1=ALU.max)
                    h2_tiles[e] = h2t

                def mm3(e, eoacc=eoacc):
                    h2t = h2_tiles.pop(e)
                    nc.tensor.matmul(
                        eoacc[:], w3s[:, e * E:(e + 1) * E], h2t[:],
                        start=(e == 0), stop=(e == E - 1))

                def make_tail(bi, gt, eoacc):
                    # Three stages, emitted between the NEXT chunk's MM1
                    # groups so the in-order PE never reaches a matmul whose
                    # ACT/DVE producer (exp, reciprocal) hasn't had a full
                    # MM1 group (~6us) to finish.
                    state = {}

                    def tail1():
                        zacc = pgt.tile([E, NBC], F32, tag="gtail")
                        for kc in range(2):
                            nc.tensor.matmul(
                                zacc[:], wg2s[:, bass.ts(kc, E)],
                                gt[:, bass.ts(kc, NBC)],
                                start=(kc == 0), stop=(kc == 1))
                        ez = tp.tile([E, NBC], F32R, tag="ez")
                        nc.scalar.activation(ez[:], zacc[:], AF.Exp, bias=bg2s[:])
                        state["ez"] = ez

                    def tail2():
                        ez = state["ez"]
                        ssum = pgt.tile([1, NBC], F32, tag="gtail")
                        nc.tensor.matmul(ssum[:], ones10[:], ez[:],
                                         start=True, stop=True)
                        rs = tp.tile([1, NBC], F32R, tag="rs")
                        with nc.allow_low_precision(reason="f32r softmax recip"):
                            nc.vector.reciprocal(rs[:], ssum[:])
                        state["rs"] = rs

                    def tail3():
                        ez, rs = state["ez"], state["rs"]
                        rbacc = pgt.tile([E, NBC], F32, tag="gtail")
                        nc.tensor.matmul(rbacc[:], ones1x10[:], rs[:],
                                         start=True, stop=True)
                        gst = tp.tile([E, NBC], F32, tag="gst")
                        nc.vector.tensor_mul(gst[:], ez[:].bitcast(F32), rbacc[:])
                        eot = tp.tile([E, NBC], F32, tag="eot")
                        nc.vector.tensor_scalar(eot[:], eoacc[:], b3s[:], None,
                                                op0=ALU.add)
                        outt = tp.tile([E, NBC], F32, tag="outt")
                        nc.vector.tensor_mul(outt[:], gst[:], eot[:])
                        nc.sync.dma_start(gs_d[:, bass.ts(bi, NBC)], gst[:])
                        nc.sync.dma_start(eo_d[:, bass.ts(bi, NBC)], eot[:])
                        nc.sync.dma_start(out_d[:, bass.ts(bi, NBC)], outt[:])

                    return (tail1, tail2, tail3)

                mm1group([E, 0])      # gate first so g is ready early
                if prev_tail is not None:
                    prev_tail[0]()
                mm1group([1, 2])
                if prev_tail is not None:
                    prev_tail[1]()
                mm2(0)
                mm1group([3, 4])
                if prev_tail is not None:
                    prev_tail[2]()
                mm2(1)
                mm2(2)
                mm3(0)
                mm1group([5, 6])
                mm2(3)
                mm3(1)
                mm2(4)
                mm3(2)
                mm1group([7, 8])
                mm2(5)
                mm3(3)
                mm2(6)
                mm3(4)
                mm1group([9])
                mm2(7)
                mm3(5)
                mm2(8)
                mm3(6)
                mm2(9)
                mm3(7)
                mm3(8)
                mm3(9)
                prev_tail = make_tail(bi, h_tiles.pop(E), eoacc)
                if bi == nb - 1:
                    prev_tail[0]()
                    prev_tail[1]()
                    prev_tail[2]()
                    prev_tail = None
            _stk.close()

    nc.compile()
    return nc


def _pack_weights(W1, b1, W2, b2, W3, b3, Wg1, bg1, Wg2, bg2):
    f32 = np.float32
    W1g = np.concatenate([W1, Wg1[None]], axis=0).astype(f32)      # [11,784,256]
    b1g = np.concatenate([b1, bg1[None]], axis=0).astype(f32)      # [11,256]
    w1p = np.ascontiguousarray(
        W1g[:, :KC1 * KP1].reshape(NU, KC1, KP1, H1)
        .transpose(2, 1, 0, 3).reshape(KP1, -1))
    w1r = np.zeros((128, NU * 2 * 128), f32)
    for u in range(NU):
        for mc in range(2):
            cp = CHAINPOS[(u, mc)]
            w1r[32 * cp:32 * cp + KR, (u * 2 + mc) * 128:(u * 2 + mc + 1) * 128] = \
                W1g[u, KC1 * KP1:, mc * 128:(mc + 1) * 128]
    b1p = np.ascontiguousarray(
        b1g.reshape(NU, 2, 128).transpose(2, 0, 1).reshape(128, NU * 2))
    w2p = np.ascontiguousarray(
        W2.astype(f32).reshape(E, 2, 128, H2).transpose(2, 0, 1, 3).reshape(128, -1))
    b2p = np.ascontiguousarray(b2.astype(f32).T)                   # [64,10]
    # sparse W3: block e is [64,10] with only column e nonzero
    w3p = np.zeros((H2, E * E), f32)
    for e in range(E):
        w3p[:, e * E + e] = W3[e]
    b3p = b3.astype(f32)[:, None]
    wg2p = np.ascontiguousarray(
        Wg2.astype(f32).reshape(2, 128, E).transpose(1, 0, 2).reshape(128, 2 * E))
    bg2p = bg2.astype(f32)[:, None]
    return {
        "w1p": w1p, "w1r": w1r, "b1p": b1p, "w2p": w2p, "b2p": b2p, "w3p": w3p,
        "b3p": b3p, "wg2p": wg2p, "bg2p": bg2p,
        "ones10": np.ones((E, 1), f32), "ones1x10": np.ones((1, E), f32),
    }


def _pack_x(xs, timing=False):
    """xs [BC, 784] f32 -> [128, NB*3584] chunk-major packed layout.

    Per chunk: 6 main K-blocks [128,512] then the 16 remainder rows
    replicated at partition offsets 0/32/64/96 (matching CHAINPOS groups).
    """
    nb = 1 if timing else NB
    xs = xs[:nb * NBC] if timing else xs
    main = np.ascontiguousarray(
        xs[:, :KC1 * KP1].reshape(nb, NBC, KC1, 128).transpose(3, 0, 2, 1))
    rem_src = xs[:, KC1 * KP1:].reshape(nb, NBC, KR).transpose(2, 0, 1)
    rem = np.zeros((128, nb, NBC), np.float32)
    for g in range(4):
        rem[32 * g:32 * g + KR] = rem_src
    full = np.concatenate(
        [main.reshape(128, nb, KC1 * NBC), rem], axis=2)
    return np.ascontiguousarray(full.reshape(128, nb * (KC1 * NBC + NBC)))


def kernel(x, W1, b1, W2, b2, W3, b3, Wg1, bg1, Wg2, bg2, _nb=NB):
    if _nb not in _CACHE:
        _CACHE[_nb] = _build(_nb)
    nc = _CACHE[_nb]

    args = [np.asarray(a, np.float32)
            for a in (W1, b1, W2, b2, W3, b3, Wg1, bg1, Wg2, bg2)]
    wmap = _pack_weights(*args)
    x = np.asarray(x, np.float32)
    in_maps = []
    for c in range(NCORES):
        in_maps.append({"xpk": _pack_x(x[c * BC:(c + 1) * BC]), **wmap})

    res = None
    for attempt in range(3):
        try:
            res = run_bass_kernel_spmd(nc, in_maps, list(range(NCORES))).results
            break
        except Exception:
            if attempt == 2:
                raise
            import time as _t
            _t.sleep(2.0)
    out = np.concatenate([res[c]["out_T"].T for c in range(NCORES)], axis=0)
    gs = np.concatenate([res[c]["gs_T"].T for c in range(NCORES)], axis=0)
    eo = np.concatenate([res[c]["eo_T"].T for c in range(NCORES)], axis=0)
    return (out, gs, eo)
